# revision 13
# baseline (speedup 1.0000x reference)
"""CRF loss (forward-algorithm log-partition + joint LLH) on 8 Trainium2 cores.

Strategy
--------
Data parallel over batch: each of the 8 cores handles 128 batch rows.

Denominator (log-partition): the 512-step forward scan runs in *scaled
probability space* so each step is one small PE matmul plus one DVE
tensor-tensor multiply:

    gamma_{s+1} = (c*A)^T gamma_s  (*) E_{s+1},   E_s = exp(emissions_s)

A = exp(Ttt) is the 48x48 transition kernel; the overflow-control constant
c is folded into the bf16 matmul weights. The scan runs forward from s=0
and backward from s=511 simultaneously (meet in the middle at s=255/256),
stacked in one [96, 64] tile: partitions 0-47 forward, 48-95 backward,
with blockdiag(cA, (cA)^T) weights. Two such fused chains (batch columns
0-63 and 64-127) interleave to hide cross-engine latency. Emissions ship
as bf16 in a partition-major [chain, 96, tick, 64] layout so each group
DMA is 96 contiguous 4KB descriptors.

Numerator (joint LLH): only the per-core *sum* is needed (the output is a
scalar mean), so
  - the transition term collapses to dot(transitions, count_matrix) where
    the 50x50 count matrix (incl. START row / END col) is a host-side
    tags transform, and
  - the emission-at-tags term is sum(raw_em (*) onehot(tags)) computed by
    one fused DVE tensor_tensor_reduce per emission group against the
    scan's already-resident tiles (one-hot ships as bf16 in the same
    layout). Both collapse into a single PSUM accumulation.

Host does only: sharding, layout transforms, count/one-hot encoding of
tags, and the final mean over the 8 per-core scalars (all-reduce glue)
plus the +511*ln(1/c) constant.
"""

import numpy as np

B, S, T = 1024, 512, 48
TT2 = T + 2                 # 50: table side incl. START/END
NCORES = 8
BL = B // NCORES            # 128 batch rows per core
NG = 2                      # fused chains per core (64 batch cols each)
W = 64                      # batch columns per chain
HT = 256                    # tick 0 = init, ticks 1..255 = scan, meet after
GK = 32                     # ticks per emission super-tile (DMA/exp batch)
NGRP = HT // GK             # 8 groups per chain

_CACHE = {}
_TRACE = False
_NUM_ENGINE = "gpsimd"      # "vector" (DVE TTR) or "gpsimd" (fused STT)
LAST = {"exec_ns": None, "results": None, "trace": None}


def _build_module():
    from concourse import bacc
    import concourse.mybir as mybir
    import concourse.tile as tile

    f32 = mybir.dt.float32
    bf16 = mybir.dt.bfloat16

    nc = bacc.Bacc(
        "TRN2",
        target_bir_lowering=False,
        debug=False,
        enable_asserts=False,
    )

    emi = nc.dram_tensor("emi", [NG, 96, HT, W], bf16, kind="ExternalInput").ap()
    hoh = nc.dram_tensor("hoh", [NG, 96, HT, W], bf16, kind="ExternalInput").ap()
    trn = nc.dram_tensor("trn", [TT2, TT2], f32, kind="ExternalInput").ap()
    cnt = nc.dram_tensor("cnt", [TT2, TT2], f32, kind="ExternalInput").ap()
    wmat = nc.dram_tensor("wmat", [96, 96], bf16, kind="ExternalInput").ap()
    initb = nc.dram_tensor("initb", [96, 1], f32, kind="ExternalInput").ap()
    den = nc.dram_tensor("den", [1, 1], f32, kind="ExternalOutput").ap()
    num = nc.dram_tensor("num", [1, 1], f32, kind="ExternalOutput").ap()

    AF = mybir.ActivationFunctionType
    OP = mybir.AluOpType
    AX = mybir.AxisListType

    with tile.TileContext(nc) as tc:
        with (
            tc.tile_pool(name="const", bufs=1) as const,
            tc.tile_pool(name="raw", bufs=3) as rawp,
            tc.tile_pool(name="ex", bufs=3) as exp_,
            tc.tile_pool(name="oh", bufs=3) as ohp,
            tc.tile_pool(name="gam", bufs=3) as gam,
            tc.tile_pool(name="prd", bufs=2) as prd,
            tc.tile_pool(name="fin", bufs=1) as fin,
            tc.tile_pool(name="ps", bufs=2, space="PSUM") as psp,
            tc.tile_pool(name="psfin", bufs=1, space="PSUM") as psf,
        ):
            # ---- constants ----
            w_sb = const.tile([96, 96], bf16, tag="w")
            nc.sync.dma_start(out=w_sb[:], in_=wmat)
            ib_sb = const.tile([96, 1], f32, tag="ib")
            nc.sync.dma_start(out=ib_sb[:], in_=initb)
            ones48 = const.tile([48, 1], f32, tag="ones48")
            nc.vector.memset(ones48[:], 1.0)
            ones50 = const.tile([TT2, 1], f32, tag="ones50")
            nc.vector.memset(ones50[:], 1.0)
            # numerator accumulator: pac[0, k] = em-sum of the k-th group
            pac = const.tile([1, NG * NGRP], f32, tag="pac")

            raw = [[None] * NGRP for _ in range(NG)]
            ex = [[None] * NGRP for _ in range(NG)]
            oh = [[None] * NGRP for _ in range(NG)]
            nacc = {"k": 0}

            def load_raw(g, grp, split=1):
                r = rawp.tile([96, GK, W], bf16, tag=f"raw{g}")
                e = exp_.tile([96, GK, W], bf16, tag=f"ex{g}")
                sub = GK // split
                for s in range(split):
                    sl = slice(s * sub, (s + 1) * sub)
                    nc.sync.dma_start(
                        out=r[:, sl, :],
                        in_=emi[g, :, grp * GK + s * sub : grp * GK + (s + 1) * sub, :],
                    )
                    nc.scalar.activation(
                        e[:, sl, :].rearrange("p a b -> p (a b)"),
                        r[:, sl, :].rearrange("p a b -> p (a b)"),
                        AF.Exp,
                    )
                raw[g][grp] = r
                ex[g][grp] = e

            def load_h(g, grp):
                h = ohp.tile([96, GK, W], bf16, tag=f"oh{g}")
                nc.sync.dma_start(
                    out=h[:], in_=hoh[g, :, grp * GK : (grp + 1) * GK, :]
                )
                oh[g][grp] = h
                # numerator em-at-tags partial: multiply + full reduce on the
                # (otherwise idle) gpsimd engine - off the scan critical path
                k = nacc["k"]
                p = prd.tile([96, GK * W], bf16, tag="prod")
                nc.gpsimd.tensor_tensor(
                    out=p[:],
                    in0=raw[g][grp][:].rearrange("p a b -> p (a b)"),
                    in1=h[:].rearrange("p a b -> p (a b)"),
                    op=OP.mult,
                )
                nc.gpsimd.tensor_reduce(
                    pac[:, k : k + 1], p[:], axis=AX.XYZWC, op=OP.add
                )
                nacc["k"] = k + 1

            # ---- scan ----
            # group 0 raw loads are split so the first exp lands early
            for g in range(NG):
                load_raw(g, 0, split=4)
            gcur = [None, None]
            for g in range(NG):
                # init: fwd row-block = exp(em_0 + trans[START,:]),
                #       bwd row-block = exp(em_511 + trans[:,END])
                g0 = gam.tile([96, W], bf16, tag=f"g{g}")
                nc.scalar.activation(
                    g0[:], raw[g][0][:, 0, :], AF.Exp, bias=ib_sb[:], scale=1.0
                )
                gcur[g] = g0
            for g in range(NG):
                load_h(g, 0)

            for t in range(1, HT):
                grp, slot = divmod(t, GK)
                for g in range(NG):
                    if raw[g][grp] is None:
                        load_raw(g, grp)
                        load_h(g, grp)
                    ps = psp.tile([96, W], f32, tag=f"ps{g}")
                    mm = nc.tensor.matmul(
                        ps[:], w_sb[:], gcur[g][:], start=True, stop=True
                    )
                    if t > 1:
                        # weights stay resident in the PE array; only the
                        # first matmul of each chain self-loads them.
                        mm.ins.ldweights = False
                    gn = gam.tile([96, W], bf16, tag=f"g{g}")
                    nc.vector.tensor_mul(
                        out=gn[:], in0=ps[:], in1=ex[g][grp][:, slot, :]
                    )
                    gcur[g] = gn

            # ---- meet in the middle:  Z*c^511 = (cA^T f_255)^T ghat_256 ----
            zt = fin.tile([48, NG * W], f32, tag="zt")
            for g in range(NG):
                psm = psf.tile([96, W], f32, tag=f"meet{g}")
                mm = nc.tensor.matmul(
                    psm[:], w_sb[:], gcur[g][:], start=True, stop=True
                )
                mm.ins.ldweights = False
                gmv = fin.tile([48, W], bf16, tag=f"gmv{g}")
                nc.sync.dma_start(out=gmv[:], in_=gcur[g][48:96, :])
                nc.vector.tensor_mul(
                    out=zt[:, g * W : (g + 1) * W], in0=psm[0:48, :], in1=gmv[:]
                )
            psz = psf.tile([1, NG * W], f32, tag="psz")
            nc.tensor.matmul(psz[:], ones48[:], zt[:], start=True, stop=True)
            dsb = fin.tile([1, NG * W], f32, tag="dsb")
            nc.scalar.activation(dsb[:], psz[:], AF.Ln)
            dred = fin.tile([1, 1], f32, tag="dred")
            nc.vector.tensor_reduce(dred[:], dsb[:], axis=AX.X, op=OP.add)
            nc.sync.dma_start(out=den, in_=dred[:])

            # ---- numerator finalization ----
            # transitions part: dot(trn, cnt) -> per-partition sums tac[50,1]
            trn_sb = const.tile([TT2, TT2], f32, tag="trn")
            nc.sync.dma_start(out=trn_sb[:], in_=trn)
            cnt_sb = const.tile([TT2, TT2], f32, tag="cnt")
            nc.sync.dma_start(out=cnt_sb[:], in_=cnt)
            tscr = fin.tile([TT2, TT2], f32, tag="tscr")
            tac = fin.tile([TT2, 1], f32, tag="tac")
            nc.vector.scalar_tensor_tensor(
                out=tscr[:],
                in0=trn_sb[:],
                scalar=1.0,
                in1=cnt_sb[:],
                op0=OP.mult,
                op1=OP.mult,
                accum_out=tac[:],
            )
            psn = psf.tile([1, 1], f32, tag="psn")
            nc.tensor.matmul(psn[:], ones50[:], tac[:], start=True, stop=True)
            # num = sum(pac) + trans dot
            pr = fin.tile([1, 2], f32, tag="pr")
            nc.vector.tensor_reduce(pr[:, 0:1], pac[:], axis=AX.X, op=OP.add)
            nc.vector.tensor_copy(out=pr[:, 1:2], in_=psn[:])
            nsb = fin.tile([1, 1], f32, tag="nsb")
            nc.vector.tensor_reduce(nsb[:], pr[:], axis=AX.X, op=OP.add)
            nc.sync.dma_start(out=num, in_=nsb[:])

    nc.compile()
    return nc


def _prep(emissions, tags, transitions):
    import ml_dtypes

    bf16 = ml_dtypes.bfloat16
    em16 = np.asarray(emissions).astype(bf16)
    tg = np.ascontiguousarray(tags).astype(np.int32)
    tr = np.ascontiguousarray(transitions, dtype=np.float32)

    A = np.exp(tr[:T, :T].astype(np.float64))
    c = 1.0 / (A.sum(axis=0).mean() * np.exp(0.5))
    logc = float(np.log(c))
    cA = c * A
    w1 = np.zeros((96, 96), np.float64)
    w1[:48, :48] = cA
    w1[48:, 48:] = cA.T
    wmat = np.ascontiguousarray(w1, dtype=bf16)
    initb = np.ascontiguousarray(
        np.concatenate([tr[T, :T], tr[:T, T + 1]])[:, None], dtype=np.float32
    )

    jj = np.arange(T, dtype=np.int32)
    mask = tg != -1
    safe = np.where(mask, tg, -2)  # never matches a real tag
    last_idx = mask.sum(axis=1) - 1

    in_maps = []
    for core in range(NCORES):
        b0 = core * BL
        em_c = em16[b0 : b0 + BL]                        # [128, 512, 48]
        tg_c = safe[b0 : b0 + BL]                        # [128, 512]
        fwd = em_c[:, :HT, :]                            # [128, 256, 48]
        bwd = em_c[:, S - 1 : HT - 1 : -1, :]            # s = 511..256
        # one-hot in [j, t, w] layout
        fH = (tg_c[:, :HT].T[None, :, :] == jj[:, None, None])
        bH = (tg_c[:, S - 1 : HT - 1 : -1].T[None, :, :] == jj[:, None, None])
        emi = np.empty((NG, 96, HT, W), bf16)
        hohv = np.empty((NG, 96, HT, W), bf16)
        for g in range(NG):
            cols = slice(g * W, (g + 1) * W)
            emi[g, :48] = fwd[cols].transpose(2, 1, 0)
            emi[g, 48:] = bwd[cols].transpose(2, 1, 0)
            hohv[g, :48] = fH[:, :, cols].astype(bf16)
            hohv[g, 48:] = bH[:, :, cols].astype(bf16)

        # transition count matrix (START row 48, END col 49)
        tgc = np.clip(tg[b0 : b0 + BL], 0, None)
        m_c = mask[b0 : b0 + BL]
        cntv = np.zeros(TT2 * TT2, np.float64)
        cntv += np.bincount(T * TT2 + tgc[:, 0], minlength=TT2 * TT2)
        pair = tgc[:, :-1] * TT2 + tgc[:, 1:]
        valid = m_c[:, 1:]
        cntv += np.bincount(pair[valid].ravel(), minlength=TT2 * TT2)
        lt = tgc[np.arange(BL), last_idx[b0 : b0 + BL]]
        cntv += np.bincount(lt * TT2 + (T + 1), minlength=TT2 * TT2)

        in_maps.append(
            {
                "emi": emi,
                "hoh": hohv,
                "trn": tr.reshape(TT2, TT2),
                "cnt": np.ascontiguousarray(
                    cntv.reshape(TT2, TT2), dtype=np.float32
                ),
                "wmat": wmat,
                "initb": initb,
            }
        )
    return in_maps, logc


def kernel(emissions, tags, transitions):
    from concourse.bass_utils import run_bass_kernel_spmd

    if "nc" not in _CACHE:
        _CACHE["nc"] = _build_module()
    nc = _CACHE["nc"]

    in_maps, logc = _prep(emissions, tags, transitions)
    res = run_bass_kernel_spmd(
        nc, in_maps, core_ids=list(range(NCORES)), trace=_TRACE
    )
    LAST["exec_ns"] = res.exec_time_ns
    LAST["results"] = res.results
    LAST["trace"] = res.instructions_and_trace

    total = 0.0
    for core in range(NCORES):
        r = res.results[core]
        d = float(r["den"].reshape(()))     # sum_b ln(c^511 * Z_b)
        n = float(r["num"].reshape(()))
        total += n - (d - BL * 511.0 * logc)
    return np.asarray(total / B, dtype=np.float32)


# revision 14
# speedup vs baseline: 1.1608x; 1.1608x over previous
"""CRF loss (forward-algorithm log-partition + joint LLH) on 8 Trainium2 cores.

Strategy
--------
Data parallel over batch: each of the 8 cores handles 128 batch rows.

Denominator (log-partition): the 512-step forward scan runs in *scaled
probability space* so each step is one small PE matmul plus one DVE
tensor-tensor multiply:

    gamma_{s+1} = (c*A)^T gamma_s  (*) E_{s+1},   E_s = exp(emissions_s)

A = exp(Ttt) is the 48x48 transition kernel; the overflow-control constant
c is folded into the bf16 matmul weights. The scan runs forward from s=0
and backward from s=511 simultaneously (meet in the middle at s=255/256),
stacked in one [96, 64] tile: partitions 0-47 forward, 48-95 backward,
with blockdiag(cA, (cA)^T) weights. Two such fused chains (batch columns
0-63 and 64-127) interleave to hide cross-engine latency. Emissions ship
as bf16 in a partition-major [chain, 96, tick, 64] layout so each group
DMA is 96 contiguous 4KB descriptors.

Numerator (joint LLH): only the per-core *sum* is needed (the output is a
scalar mean), so
  - the transition term collapses to dot(transitions, count_matrix) where
    the 50x50 count matrix (incl. START row / END col) is a host-side
    tags transform, and
  - the emission-at-tags term is sum(raw_em (*) onehot(tags)) computed by
    one fused DVE tensor_tensor_reduce per emission group against the
    scan's already-resident tiles (one-hot ships as bf16 in the same
    layout). Both collapse into a single PSUM accumulation.

Host does only: sharding, layout transforms, count/one-hot encoding of
tags, and the final mean over the 8 per-core scalars (all-reduce glue)
plus the +511*ln(1/c) constant.
"""

import numpy as np

B, S, T = 1024, 512, 48
TT2 = T + 2                 # 50: table side incl. START/END
NCORES = 8
BL = B // NCORES            # 128 batch rows per core
NG = 2                      # fused chains per core (64 batch cols each)
W = 64                      # batch columns per chain
HT = 256                    # tick 0 = init, ticks 1..255 = scan, meet after
GK = 32                     # ticks per emission super-tile (DMA/exp batch)
NGRP = HT // GK             # 8 groups per chain

_CACHE = {}
_TRACE = False
_NUM_ENGINE = "gpsimd"      # "vector" (DVE TTR) or "gpsimd" (fused STT)
LAST = {"exec_ns": None, "results": None, "trace": None}


def _build_module():
    from concourse import bacc
    import concourse.mybir as mybir
    import concourse.tile as tile

    f32 = mybir.dt.float32
    bf16 = mybir.dt.bfloat16

    nc = bacc.Bacc(
        "TRN2",
        target_bir_lowering=False,
        debug=False,
        enable_asserts=False,
    )

    emi = nc.dram_tensor("emi", [NG, 96, HT, W], bf16, kind="ExternalInput").ap()
    hoh = nc.dram_tensor("hoh", [NG, 96, HT, W], bf16, kind="ExternalInput").ap()
    trn = nc.dram_tensor("trn", [TT2, TT2], f32, kind="ExternalInput").ap()
    cnt = nc.dram_tensor("cnt", [TT2, TT2], f32, kind="ExternalInput").ap()
    wmat = nc.dram_tensor("wmat", [96, 96], bf16, kind="ExternalInput").ap()
    initb = nc.dram_tensor("initb", [96, 1], f32, kind="ExternalInput").ap()
    den = nc.dram_tensor("den", [1, 1], f32, kind="ExternalOutput").ap()
    num = nc.dram_tensor("num", [1, 1], f32, kind="ExternalOutput").ap()

    AF = mybir.ActivationFunctionType
    OP = mybir.AluOpType
    AX = mybir.AxisListType

    with tile.TileContext(nc) as tc:
        with (
            tc.tile_pool(name="const", bufs=1) as const,
            tc.tile_pool(name="raw", bufs=4) as rawp,
            tc.tile_pool(name="ex", bufs=4) as exp_,
            tc.tile_pool(name="oh", bufs=4) as ohp,
            tc.tile_pool(name="gam", bufs=3) as gam,
            tc.tile_pool(name="prd", bufs=2) as prd,
            tc.tile_pool(name="fin", bufs=1) as fin,
            tc.tile_pool(name="ps", bufs=2, space="PSUM") as psp,
            tc.tile_pool(name="psfin", bufs=1, space="PSUM") as psf,
        ):
            # ---- constants ----
            w_sb = const.tile([96, 96], bf16, tag="w")
            nc.sync.dma_start(out=w_sb[:], in_=wmat)
            ib_sb = const.tile([96, 1], f32, tag="ib")
            nc.sync.dma_start(out=ib_sb[:], in_=initb)
            ones48 = const.tile([48, 1], f32, tag="ones48")
            nc.vector.memset(ones48[:], 1.0)
            ones96 = const.tile([96, 1], f32, tag="ones96")
            nc.vector.memset(ones96[:], 1.0)
            ones50 = const.tile([TT2, 1], f32, tag="ones50")
            nc.vector.memset(ones50[:], 1.0)
            # numerator accumulators: gpsimd groups write pac[0, k];
            # DVE vstt groups write pacv[:, j] (per-partition sums).
            # pac cols NG*NGRP and NG*NGRP+1 take the two PSUM dots at the end.
            pac = const.tile([1, NG * NGRP + 2], f32, tag="pac")
            nc.vector.memset(pac[:], 0.0)
            DVE_KS = (2, 5, 8, 11, 14, 15)
            pacv = const.tile([96, len(DVE_KS)], f32, tag="pacv")
            nc.vector.memset(pacv[:], 0.0)

            raw = [[None] * NGRP for _ in range(NG)]
            ex = [[None] * NGRP for _ in range(NG)]
            oh = [[None] * NGRP for _ in range(NG)]
            nacc = {"k": 0}

            def load_raw(g, grp, split=1):
                r = rawp.tile([96, GK, W], bf16, tag=f"raw{g}")
                e = exp_.tile([96, GK, W], bf16, tag=f"ex{g}")
                sub = GK // split
                for s in range(split):
                    sl = slice(s * sub, (s + 1) * sub)
                    nc.sync.dma_start(
                        out=r[:, sl, :],
                        in_=emi[g, :, grp * GK + s * sub : grp * GK + (s + 1) * sub, :],
                    )
                    nc.scalar.activation(
                        e[:, sl, :].rearrange("p a b -> p (a b)"),
                        r[:, sl, :].rearrange("p a b -> p (a b)"),
                        AF.Exp,
                    )
                raw[g][grp] = r
                ex[g][grp] = e

            def load_h(g, grp):
                h = ohp.tile([96, GK, W], bf16, tag=f"oh{g}")
                nc.sync.dma_start(
                    out=h[:], in_=hoh[g, :, grp * GK : (grp + 1) * GK, :]
                )
                oh[g][grp] = h
                # numerator em-at-tags partial: most groups run multiply +
                # full reduce on the (otherwise idle) gpsimd engine; a few
                # run as one fused DVE op so gpsimd stays under the scan span
                k = nacc["k"]
                p = prd.tile([96, GK * W], bf16, tag="prod")
                if k in DVE_KS:
                    j = DVE_KS.index(k)
                    nc.vector.scalar_tensor_tensor(
                        out=p[:],
                        in0=raw[g][grp][:].rearrange("p a b -> p (a b)"),
                        scalar=1.0,
                        in1=h[:].rearrange("p a b -> p (a b)"),
                        op0=OP.mult,
                        op1=OP.mult,
                        accum_out=pacv[:, j : j + 1],
                    )
                else:
                    nc.gpsimd.tensor_tensor(
                        out=p[:],
                        in0=raw[g][grp][:].rearrange("p a b -> p (a b)"),
                        in1=h[:].rearrange("p a b -> p (a b)"),
                        op=OP.mult,
                    )
                    nc.gpsimd.tensor_reduce(
                        pac[:, k : k + 1], p[:], axis=AX.XYZWC, op=OP.add
                    )
                nacc["k"] = k + 1

            # ---- scan ----
            # group 0 raw loads are split so the first exp lands early
            for g in range(NG):
                load_raw(g, 0, split=4)
            gcur = [None, None]
            for g in range(NG):
                # init: fwd row-block = exp(em_0 + trans[START,:]),
                #       bwd row-block = exp(em_511 + trans[:,END])
                g0 = gam.tile([96, W], bf16, tag=f"g{g}")
                nc.scalar.activation(
                    g0[:], raw[g][0][:, 0, :], AF.Exp, bias=ib_sb[:], scale=1.0
                )
                gcur[g] = g0
            for g in range(NG):
                load_h(g, 0)
            # small tail-phase constants: post now so their completion is not
            # queued behind the whole emission stream
            trn_sb = const.tile([TT2, TT2], f32, tag="trn")
            nc.sync.dma_start(out=trn_sb[:], in_=trn)
            cnt_sb = const.tile([TT2, TT2], f32, tag="cnt")
            nc.sync.dma_start(out=cnt_sb[:], in_=cnt)

            for t in range(1, HT):
                grp, slot = divmod(t, GK)
                for g in range(NG):
                    if raw[g][grp] is None:
                        load_raw(g, grp)
                        load_h(g, grp)
                    ps = psp.tile([96, W], f32, tag=f"ps{g}")
                    mm = nc.tensor.matmul(
                        ps[:], w_sb[:], gcur[g][:], start=True, stop=True
                    )
                    if t > 1:
                        # weights stay resident in the PE array; only the
                        # first matmul of each chain self-loads them.
                        mm.ins.ldweights = False
                    gn = gam.tile([96, W], bf16, tag=f"g{g}")
                    nc.vector.tensor_mul(
                        out=gn[:], in0=ps[:], in1=ex[g][grp][:, slot, :]
                    )
                    gcur[g] = gn

            # ---- meet in the middle:  Z*c^511 = (cA^T f_255)^T ghat_256 ----
            zt = fin.tile([48, NG * W], f32, tag="zt")
            for g in range(NG):
                psm = psf.tile([96, W], f32, tag=f"meet{g}")
                mm = nc.tensor.matmul(
                    psm[:], w_sb[:], gcur[g][:], start=True, stop=True
                )
                mm.ins.ldweights = False
                gmv = fin.tile([48, W], bf16, tag=f"gmv{g}")
                nc.sync.dma_start(out=gmv[:], in_=gcur[g][48:96, :])
                nc.vector.tensor_mul(
                    out=zt[:, g * W : (g + 1) * W], in0=psm[0:48, :], in1=gmv[:]
                )
            psz = psf.tile([1, NG * W], f32, tag="psz")
            nc.tensor.matmul(psz[:], ones48[:], zt[:], start=True, stop=True)
            dsb = fin.tile([1, NG * W], f32, tag="dsb")
            nc.scalar.activation(dsb[:], psz[:], AF.Ln)
            dred = fin.tile([1, 1], f32, tag="dred")
            nc.vector.tensor_reduce(dred[:], dsb[:], axis=AX.X, op=OP.add)
            nc.sync.dma_start(out=den, in_=dred[:])

            # ---- numerator finalization ----
            # transitions part: dot(trn, cnt) -> per-partition sums tac[50,1]
            tscr = fin.tile([TT2, TT2], f32, tag="tscr")
            tac = fin.tile([TT2, 1], f32, tag="tac")
            nc.vector.scalar_tensor_tensor(
                out=tscr[:],
                in0=trn_sb[:],
                scalar=1.0,
                in1=cnt_sb[:],
                op0=OP.mult,
                op1=OP.mult,
                accum_out=tac[:],
            )
            pvr = fin.tile([96, 1], f32, tag="pvr")
            nc.vector.tensor_reduce(pvr[:], pacv[:], axis=AX.X, op=OP.add)
            psn = psf.tile([1, 2], f32, tag="psn")
            nc.tensor.matmul(psn[:, 0:1], ones96[:], pvr[:], start=True, stop=True)
            nc.tensor.matmul(psn[:, 1:2], ones50[:], tac[:], start=True, stop=True)
            # num = sum of gpsimd group sums + DVE group sums + trans dot
            nc.vector.tensor_copy(out=pac[:, NG * NGRP : NG * NGRP + 2], in_=psn[:])
            nsb = fin.tile([1, 1], f32, tag="nsb")
            nc.vector.tensor_reduce(nsb[:], pac[:], axis=AX.X, op=OP.add)
            nc.sync.dma_start(out=num, in_=nsb[:])

    nc.compile()
    return nc


def _prep(emissions, tags, transitions):
    import ml_dtypes

    bf16 = ml_dtypes.bfloat16
    em16 = np.asarray(emissions).astype(bf16)
    tg = np.ascontiguousarray(tags).astype(np.int32)
    tr = np.ascontiguousarray(transitions, dtype=np.float32)

    A = np.exp(tr[:T, :T].astype(np.float64))
    c = 1.0 / (A.sum(axis=0).mean() * np.exp(0.5))
    logc = float(np.log(c))
    cA = c * A
    w1 = np.zeros((96, 96), np.float64)
    w1[:48, :48] = cA
    w1[48:, 48:] = cA.T
    wmat = np.ascontiguousarray(w1, dtype=bf16)
    initb = np.ascontiguousarray(
        np.concatenate([tr[T, :T], tr[:T, T + 1]])[:, None], dtype=np.float32
    )

    jj = np.arange(T, dtype=np.int32)
    mask = tg != -1
    safe = np.where(mask, tg, -2)  # never matches a real tag
    last_idx = mask.sum(axis=1) - 1

    in_maps = []
    for core in range(NCORES):
        b0 = core * BL
        em_c = em16[b0 : b0 + BL]                        # [128, 512, 48]
        tg_c = safe[b0 : b0 + BL]                        # [128, 512]
        fwd = em_c[:, :HT, :]                            # [128, 256, 48]
        bwd = em_c[:, S - 1 : HT - 1 : -1, :]            # s = 511..256
        # one-hot in [j, t, w] layout
        fH = (tg_c[:, :HT].T[None, :, :] == jj[:, None, None])
        bH = (tg_c[:, S - 1 : HT - 1 : -1].T[None, :, :] == jj[:, None, None])
        emi = np.empty((NG, 96, HT, W), bf16)
        hohv = np.empty((NG, 96, HT, W), bf16)
        for g in range(NG):
            cols = slice(g * W, (g + 1) * W)
            emi[g, :48] = fwd[cols].transpose(2, 1, 0)
            emi[g, 48:] = bwd[cols].transpose(2, 1, 0)
            hohv[g, :48] = fH[:, :, cols].astype(bf16)
            hohv[g, 48:] = bH[:, :, cols].astype(bf16)

        # transition count matrix (START row 48, END col 49)
        tgc = np.clip(tg[b0 : b0 + BL], 0, None)
        m_c = mask[b0 : b0 + BL]
        cntv = np.zeros(TT2 * TT2, np.float64)
        cntv += np.bincount(T * TT2 + tgc[:, 0], minlength=TT2 * TT2)
        pair = tgc[:, :-1] * TT2 + tgc[:, 1:]
        valid = m_c[:, 1:]
        cntv += np.bincount(pair[valid].ravel(), minlength=TT2 * TT2)
        lt = tgc[np.arange(BL), last_idx[b0 : b0 + BL]]
        cntv += np.bincount(lt * TT2 + (T + 1), minlength=TT2 * TT2)

        in_maps.append(
            {
                "emi": emi,
                "hoh": hohv,
                "trn": tr.reshape(TT2, TT2),
                "cnt": np.ascontiguousarray(
                    cntv.reshape(TT2, TT2), dtype=np.float32
                ),
                "wmat": wmat,
                "initb": initb,
            }
        )
    return in_maps, logc


def kernel(emissions, tags, transitions):
    from concourse.bass_utils import run_bass_kernel_spmd

    if "nc" not in _CACHE:
        _CACHE["nc"] = _build_module()
    nc = _CACHE["nc"]

    in_maps, logc = _prep(emissions, tags, transitions)
    res = run_bass_kernel_spmd(
        nc, in_maps, core_ids=list(range(NCORES)), trace=_TRACE
    )
    LAST["exec_ns"] = res.exec_time_ns
    LAST["results"] = res.results
    LAST["trace"] = res.instructions_and_trace

    total = 0.0
    for core in range(NCORES):
        r = res.results[core]
        d = float(r["den"].reshape(()))     # sum_b ln(c^511 * Z_b)
        n = float(r["num"].reshape(()))
        total += n - (d - BL * 511.0 * logc)
    return np.asarray(total / B, dtype=np.float32)


# revision 15
# speedup vs baseline: 1.7216x; 1.4831x over previous
"""CRF loss (forward-algorithm log-partition + joint LLH) on 8 Trainium2 cores.

Strategy
--------
Data parallel over batch: each of the 8 cores handles 128 batch rows.

Denominator (log-partition): the 512-step forward scan runs in *scaled
probability space* so each step is one small PE matmul plus one DVE
tensor-tensor multiply:

    gamma_{s+1} = (c*A)^T gamma_s  (*) E_{s+1},   E_s = exp(emissions_s)

A = exp(Ttt) is the 48x48 transition kernel; the overflow-control constant
c is folded into the bf16 matmul weights. The scan runs forward from s=0
and backward from s=511 simultaneously (meet in the middle at s=255/256),
stacked in one [96, 64] tile: partitions 0-47 forward, 48-95 backward,
with blockdiag(cA, (cA)^T) weights. Two such fused chains (batch columns
0-63 and 64-127) interleave to hide cross-engine latency. Emissions ship
as bf16 in a partition-major [chain, 96, tick, 64] layout so each group
DMA is 96 contiguous 4KB descriptors.

Numerator (joint LLH): only the per-core *sum* is needed (the output is a
scalar mean), so
  - the transition term collapses to dot(transitions, count_matrix) where
    the 50x50 count matrix (incl. START row / END col) is a host-side
    tags transform, and
  - the emission-at-tags term is sum(raw_em (*) onehot(tags)) computed by
    one fused DVE tensor_tensor_reduce per emission group against the
    scan's already-resident tiles (one-hot ships as bf16 in the same
    layout). Both collapse into a single PSUM accumulation.

Host does only: sharding, layout transforms, count/one-hot encoding of
tags, and the final mean over the 8 per-core scalars (all-reduce glue)
plus the +511*ln(1/c) constant.
"""

import numpy as np

B, S, T = 1024, 512, 48
TT2 = T + 2                 # 50: table side incl. START/END
NCORES = 8
BL = B // NCORES            # 128 batch rows per core
NG = 2                      # fused chains per core (64 batch cols each)
W = 64                      # batch columns per chain
HT = 256                    # tick 0 = init, ticks 1..255 = scan, meet after
GK = 32                     # ticks per emission super-tile (DMA/exp batch)
NGRP = HT // GK             # 8 groups per chain

_CACHE = {}
_TRACE = False
_NUM_ENGINE = "gpsimd"      # "vector" (DVE TTR) or "gpsimd" (fused STT)
LAST = {"exec_ns": None, "results": None, "trace": None}


def _build_module():
    from concourse import bacc
    import concourse.mybir as mybir
    import concourse.tile as tile

    f32 = mybir.dt.float32
    bf16 = mybir.dt.bfloat16

    nc = bacc.Bacc(
        "TRN2",
        target_bir_lowering=False,
        debug=False,
        enable_asserts=False,
    )

    emi = nc.dram_tensor("emi", [NG, 96, HT, W], bf16, kind="ExternalInput").ap()
    hoh = nc.dram_tensor("hoh", [NG, 96, HT, W], bf16, kind="ExternalInput").ap()
    trn = nc.dram_tensor("trn", [TT2, TT2], f32, kind="ExternalInput").ap()
    cnt = nc.dram_tensor("cnt", [TT2, TT2], f32, kind="ExternalInput").ap()
    wmat = nc.dram_tensor("wmat", [96, 96], bf16, kind="ExternalInput").ap()
    initb = nc.dram_tensor("initb", [96, 1], f32, kind="ExternalInput").ap()
    den = nc.dram_tensor("den", [1, 1], f32, kind="ExternalOutput").ap()
    num = nc.dram_tensor("num", [1, 1], f32, kind="ExternalOutput").ap()

    AF = mybir.ActivationFunctionType
    OP = mybir.AluOpType
    AX = mybir.AxisListType

    with tile.TileContext(nc) as tc:
        with (
            tc.tile_pool(name="const", bufs=1) as const,
            tc.tile_pool(name="raw", bufs=4) as rawp,
            tc.tile_pool(name="ex", bufs=4) as exp_,
            tc.tile_pool(name="oh", bufs=4) as ohp,
            tc.tile_pool(name="gam", bufs=3) as gam,
            tc.tile_pool(name="prd", bufs=2) as prd,
            tc.tile_pool(name="scr", bufs=2) as scrp,
            tc.tile_pool(name="fin", bufs=1) as fin,
            tc.tile_pool(name="ps", bufs=2, space="PSUM") as psp,
            tc.tile_pool(name="psfin", bufs=1, space="PSUM") as psf,
        ):
            # ---- constants ----
            w_sb = const.tile([96, 96], bf16, tag="w")
            nc.sync.dma_start(out=w_sb[:], in_=wmat)
            ib_sb = const.tile([96, 1], f32, tag="ib")
            nc.sync.dma_start(out=ib_sb[:], in_=initb)
            ones48 = const.tile([48, 1], f32, tag="ones48")
            nc.vector.memset(ones48[:], 1.0)
            ones96 = const.tile([96, 1], f32, tag="ones96")
            nc.vector.memset(ones96[:], 1.0)
            ones50 = const.tile([TT2, 1], f32, tag="ones50")
            nc.vector.memset(ones50[:], 1.0)
            # numerator accumulator: pacs[:, k] = per-partition em-sums of
            # group k (gpsimd multiplies, scalar engine reduces via accum_out)
            pacs = const.tile([96, NG * NGRP], f32, tag="pacs")

            raw = [[None] * NGRP for _ in range(NG)]
            ex = [[None] * NGRP for _ in range(NG)]
            oh = [[None] * NGRP for _ in range(NG)]
            nacc = {"k": 0}

            def load_raw(g, grp, split=1):
                r = rawp.tile([96, GK, W], bf16, tag=f"raw{g}")
                e = exp_.tile([96, GK, W], bf16, tag=f"ex{g}")
                sub = GK // split
                for s in range(split):
                    sl = slice(s * sub, (s + 1) * sub)
                    nc.sync.dma_start(
                        out=r[:, sl, :],
                        in_=emi[g, :, grp * GK + s * sub : grp * GK + (s + 1) * sub, :],
                    )
                    nc.scalar.activation(
                        e[:, sl, :].rearrange("p a b -> p (a b)"),
                        r[:, sl, :].rearrange("p a b -> p (a b)"),
                        AF.Exp,
                    )
                raw[g][grp] = r
                ex[g][grp] = e

            def load_h(g, grp):
                h = ohp.tile([96, GK, W], bf16, tag=f"oh{g}")
                nc.sync.dma_start(
                    out=h[:], in_=hoh[g, :, grp * GK : (grp + 1) * GK, :]
                )
                oh[g][grp] = h
                # numerator em-at-tags partial: multiply on the idle gpsimd,
                # free-axis reduce on the idle scalar engine (activation
                # accumulate) - the scan-critical DVE is untouched
                k = nacc["k"]
                p = prd.tile([96, GK * W], bf16, tag="prod")
                nc.gpsimd.tensor_tensor(
                    out=p[:],
                    in0=raw[g][grp][:].rearrange("p a b -> p (a b)"),
                    in1=h[:].rearrange("p a b -> p (a b)"),
                    op=OP.mult,
                )
                scr = scrp.tile([96, GK * W], bf16, tag="scr")
                nc.scalar.activation(
                    scr[:], p[:], AF.Identity, accum_out=pacs[:, k : k + 1]
                )
                nacc["k"] = k + 1

            # ---- scan ----
            # group 0 raw loads are split so the first exp lands early
            for g in range(NG):
                load_raw(g, 0, split=4)
            gcur = [None, None]
            for g in range(NG):
                # init: fwd row-block = exp(em_0 + trans[START,:]),
                #       bwd row-block = exp(em_511 + trans[:,END])
                g0 = gam.tile([96, W], bf16, tag=f"g{g}")
                nc.scalar.activation(
                    g0[:], raw[g][0][:, 0, :], AF.Exp, bias=ib_sb[:], scale=1.0
                )
                gcur[g] = g0
            for g in range(NG):
                load_h(g, 0)
            # small tail-phase constants: post now so their completion is not
            # queued behind the whole emission stream
            trn_sb = const.tile([TT2, TT2], f32, tag="trn")
            nc.sync.dma_start(out=trn_sb[:], in_=trn)
            cnt_sb = const.tile([TT2, TT2], f32, tag="cnt")
            nc.sync.dma_start(out=cnt_sb[:], in_=cnt)

            for t in range(1, HT):
                grp, slot = divmod(t, GK)
                for g in range(NG):
                    if raw[g][grp] is None:
                        load_raw(g, grp)
                        load_h(g, grp)
                    ps = psp.tile([96, W], f32, tag=f"ps{g}")
                    mm = nc.tensor.matmul(
                        ps[:], w_sb[:], gcur[g][:], start=True, stop=True
                    )
                    if t > 1:
                        # weights stay resident in the PE array; only the
                        # first matmul of each chain self-loads them.
                        mm.ins.ldweights = False
                    gn = gam.tile([96, W], bf16, tag=f"g{g}")
                    nc.vector.tensor_mul(
                        out=gn[:], in0=ps[:], in1=ex[g][grp][:, slot, :]
                    )
                    gcur[g] = gn

            # ---- meet in the middle:  Z*c^511 = (cA^T f_255)^T ghat_256 ----
            zt = fin.tile([48, NG * W], f32, tag="zt")
            for g in range(NG):
                psm = psf.tile([96, W], f32, tag=f"meet{g}")
                mm = nc.tensor.matmul(
                    psm[:], w_sb[:], gcur[g][:], start=True, stop=True
                )
                mm.ins.ldweights = False
                gmv = fin.tile([48, W], bf16, tag=f"gmv{g}")
                nc.sync.dma_start(out=gmv[:], in_=gcur[g][48:96, :])
                nc.vector.tensor_mul(
                    out=zt[:, g * W : (g + 1) * W], in0=psm[0:48, :], in1=gmv[:]
                )
            psz = psf.tile([1, NG * W], f32, tag="psz")
            nc.tensor.matmul(psz[:], ones48[:], zt[:], start=True, stop=True)
            dsb = fin.tile([1, NG * W], f32, tag="dsb")
            nc.scalar.activation(dsb[:], psz[:], AF.Ln)
            dred = fin.tile([1, 1], f32, tag="dred")
            nc.vector.tensor_reduce(dred[:], dsb[:], axis=AX.X, op=OP.add)
            nc.sync.dma_start(out=den, in_=dred[:])

            # ---- numerator finalization ----
            # transitions part: dot(trn, cnt) -> per-partition sums tac[50,1]
            tscr = fin.tile([TT2, TT2], f32, tag="tscr")
            tac = fin.tile([TT2, 1], f32, tag="tac")
            nc.vector.scalar_tensor_tensor(
                out=tscr[:],
                in0=trn_sb[:],
                scalar=1.0,
                in1=cnt_sb[:],
                op0=OP.mult,
                op1=OP.mult,
                accum_out=tac[:],
            )
            pvr = fin.tile([96, 1], f32, tag="pvr")
            nc.vector.tensor_reduce(pvr[:], pacs[:], axis=AX.X, op=OP.add)
            psn = psf.tile([1, 2], f32, tag="psn")
            nc.tensor.matmul(psn[:, 0:1], ones96[:], pvr[:], start=True, stop=True)
            nc.tensor.matmul(psn[:, 1:2], ones50[:], tac[:], start=True, stop=True)
            # num = em-at-tags total + transitions dot
            pr = fin.tile([1, 2], f32, tag="pr")
            nc.vector.tensor_copy(out=pr[:], in_=psn[:])
            nsb = fin.tile([1, 1], f32, tag="nsb")
            nc.vector.tensor_reduce(nsb[:], pr[:], axis=AX.X, op=OP.add)
            nc.sync.dma_start(out=num, in_=nsb[:])

    nc.compile()
    return nc


def _prep(emissions, tags, transitions):
    import ml_dtypes

    bf16 = ml_dtypes.bfloat16
    em16 = np.asarray(emissions).astype(bf16)
    tg = np.ascontiguousarray(tags).astype(np.int32)
    tr = np.ascontiguousarray(transitions, dtype=np.float32)

    A = np.exp(tr[:T, :T].astype(np.float64))
    c = 1.0 / (A.sum(axis=0).mean() * np.exp(0.5))
    logc = float(np.log(c))
    cA = c * A
    w1 = np.zeros((96, 96), np.float64)
    w1[:48, :48] = cA
    w1[48:, 48:] = cA.T
    wmat = np.ascontiguousarray(w1, dtype=bf16)
    initb = np.ascontiguousarray(
        np.concatenate([tr[T, :T], tr[:T, T + 1]])[:, None], dtype=np.float32
    )

    jj = np.arange(T, dtype=np.int32)
    mask = tg != -1
    safe = np.where(mask, tg, -2)  # never matches a real tag
    last_idx = mask.sum(axis=1) - 1

    in_maps = []
    for core in range(NCORES):
        b0 = core * BL
        em_c = em16[b0 : b0 + BL]                        # [128, 512, 48]
        tg_c = safe[b0 : b0 + BL]                        # [128, 512]
        fwd = em_c[:, :HT, :]                            # [128, 256, 48]
        bwd = em_c[:, S - 1 : HT - 1 : -1, :]            # s = 511..256
        # one-hot in [j, t, w] layout
        fH = (tg_c[:, :HT].T[None, :, :] == jj[:, None, None])
        bH = (tg_c[:, S - 1 : HT - 1 : -1].T[None, :, :] == jj[:, None, None])
        emi = np.empty((NG, 96, HT, W), bf16)
        hohv = np.empty((NG, 96, HT, W), bf16)
        for g in range(NG):
            cols = slice(g * W, (g + 1) * W)
            emi[g, :48] = fwd[cols].transpose(2, 1, 0)
            emi[g, 48:] = bwd[cols].transpose(2, 1, 0)
            hohv[g, :48] = fH[:, :, cols].astype(bf16)
            hohv[g, 48:] = bH[:, :, cols].astype(bf16)

        # transition count matrix (START row 48, END col 49)
        tgc = np.clip(tg[b0 : b0 + BL], 0, None)
        m_c = mask[b0 : b0 + BL]
        cntv = np.zeros(TT2 * TT2, np.float64)
        cntv += np.bincount(T * TT2 + tgc[:, 0], minlength=TT2 * TT2)
        pair = tgc[:, :-1] * TT2 + tgc[:, 1:]
        valid = m_c[:, 1:]
        cntv += np.bincount(pair[valid].ravel(), minlength=TT2 * TT2)
        lt = tgc[np.arange(BL), last_idx[b0 : b0 + BL]]
        cntv += np.bincount(lt * TT2 + (T + 1), minlength=TT2 * TT2)

        in_maps.append(
            {
                "emi": emi,
                "hoh": hohv,
                "trn": tr.reshape(TT2, TT2),
                "cnt": np.ascontiguousarray(
                    cntv.reshape(TT2, TT2), dtype=np.float32
                ),
                "wmat": wmat,
                "initb": initb,
            }
        )
    return in_maps, logc


def kernel(emissions, tags, transitions):
    from concourse.bass_utils import run_bass_kernel_spmd

    if "nc" not in _CACHE:
        _CACHE["nc"] = _build_module()
    nc = _CACHE["nc"]

    in_maps, logc = _prep(emissions, tags, transitions)
    res = run_bass_kernel_spmd(
        nc, in_maps, core_ids=list(range(NCORES)), trace=_TRACE
    )
    LAST["exec_ns"] = res.exec_time_ns
    LAST["results"] = res.results
    LAST["trace"] = res.instructions_and_trace

    total = 0.0
    for core in range(NCORES):
        r = res.results[core]
        d = float(r["den"].reshape(()))     # sum_b ln(c^511 * Z_b)
        n = float(r["num"].reshape(()))
        total += n - (d - BL * 511.0 * logc)
    return np.asarray(total / B, dtype=np.float32)


# revision 17
# speedup vs baseline: 1.7530x; 1.0183x over previous
"""CRF loss (forward-algorithm log-partition + joint LLH) on 8 Trainium2 cores.

Strategy
--------
Data parallel over batch: each of the 8 cores handles 128 batch rows.

Denominator (log-partition): the 512-step forward scan runs in *scaled
probability space* so each step is one small PE matmul plus one DVE
tensor-tensor multiply:

    gamma_{s+1} = (c*A)^T gamma_s  (*) E_{s+1},   E_s = exp(emissions_s)

A = exp(Ttt) is the 48x48 transition kernel; the overflow-control constant
c is folded into the bf16 matmul weights. The scan runs forward from s=0
and backward from s=511 simultaneously (meet in the middle at s=255/256),
stacked in one [96, 64] tile: partitions 0-47 forward, 48-95 backward,
with blockdiag(cA, (cA)^T) weights. Two such fused chains (batch columns
0-63 and 64-127) interleave to hide cross-engine latency. Emissions ship
as bf16 in a partition-major [chain, 96, tick, 64] layout so each group
DMA is 96 contiguous 4KB descriptors.

Numerator (joint LLH): only the per-core *sum* is needed (the output is a
scalar mean), so
  - the transition term collapses to dot(transitions, count_matrix) where
    the 50x50 count matrix (incl. START row / END col) is a host-side
    tags transform, and
  - the emission-at-tags term is sum(raw_em (*) onehot(tags)) computed by
    one fused DVE tensor_tensor_reduce per emission group against the
    scan's already-resident tiles (one-hot ships as bf16 in the same
    layout). Both collapse into a single PSUM accumulation.

Host does only: sharding, layout transforms, count/one-hot encoding of
tags, and the final mean over the 8 per-core scalars (all-reduce glue)
plus the +511*ln(1/c) constant.
"""

import numpy as np

B, S, T = 1024, 512, 48
TT2 = T + 2                 # 50: table side incl. START/END
NCORES = 8
BL = B // NCORES            # 128 batch rows per core
NG = 2                      # fused chains per core (64 batch cols each)
W = 64                      # batch columns per chain
HT = 256                    # tick 0 = init, ticks 1..255 = scan, meet after
GK = 32                     # ticks per emission super-tile (DMA/exp batch)
NGRP = HT // GK             # 8 groups per chain

_CACHE = {}
_TRACE = False
_NUM_ENGINE = "gpsimd"      # "vector" (DVE TTR) or "gpsimd" (fused STT)
LAST = {"exec_ns": None, "results": None, "trace": None}


def _build_module():
    from concourse import bacc
    import concourse.mybir as mybir
    import concourse.tile as tile

    f32 = mybir.dt.float32
    bf16 = mybir.dt.bfloat16

    nc = bacc.Bacc(
        "TRN2",
        target_bir_lowering=False,
        debug=False,
        enable_asserts=False,
    )

    emi = nc.dram_tensor("emi", [NG, 96, HT, W], bf16, kind="ExternalInput").ap()
    hoh = nc.dram_tensor("hoh", [NG, 96, HT, W], bf16, kind="ExternalInput").ap()
    trn = nc.dram_tensor("trn", [TT2, TT2], f32, kind="ExternalInput").ap()
    cnt = nc.dram_tensor("cnt", [TT2, TT2], f32, kind="ExternalInput").ap()
    wmat = nc.dram_tensor("wmat", [96, 96], bf16, kind="ExternalInput").ap()
    initb = nc.dram_tensor("initb", [96, 1], f32, kind="ExternalInput").ap()
    den = nc.dram_tensor("den", [1, 1], f32, kind="ExternalOutput").ap()
    num = nc.dram_tensor("num", [1, 1], f32, kind="ExternalOutput").ap()

    AF = mybir.ActivationFunctionType
    OP = mybir.AluOpType
    AX = mybir.AxisListType

    with tile.TileContext(nc) as tc:
        with (
            tc.tile_pool(name="const", bufs=1) as const,
            tc.tile_pool(name="raw", bufs=4) as rawp,
            tc.tile_pool(name="ex", bufs=4) as exp_,
            tc.tile_pool(name="oh", bufs=4) as ohp,
            tc.tile_pool(name="gam", bufs=3) as gam,
            tc.tile_pool(name="prd", bufs=3) as prd,
            tc.tile_pool(name="scr", bufs=2) as scrp,
            tc.tile_pool(name="fin", bufs=1) as fin,
            tc.tile_pool(name="ps", bufs=2, space="PSUM") as psp,
            tc.tile_pool(name="psfin", bufs=1, space="PSUM") as psf,
        ):
            # ---- constants ----
            w_sb = const.tile([96, 96], bf16, tag="w")
            nc.sync.dma_start(out=w_sb[:], in_=wmat)
            ib_sb = const.tile([96, 1], f32, tag="ib")
            nc.sync.dma_start(out=ib_sb[:], in_=initb)
            ones48 = const.tile([48, 1], f32, tag="ones48")
            nc.vector.memset(ones48[:], 1.0)
            ones96 = const.tile([96, 1], f32, tag="ones96")
            nc.vector.memset(ones96[:], 1.0)
            ones50 = const.tile([TT2, 1], f32, tag="ones50")
            nc.vector.memset(ones50[:], 1.0)
            # numerator accumulator: pacs[:, k] = per-partition em-sums of
            # group k (gpsimd multiplies, scalar engine reduces via accum_out)
            pacs = const.tile([96, NG * NGRP], f32, tag="pacs")

            raw = [[None] * NGRP for _ in range(NG)]
            ex = [[None] * NGRP for _ in range(NG)]
            oh = [[None] * NGRP for _ in range(NG)]
            nacc = {"k": 0}
            pend = []

            def load_raw(g, grp, split=1, on_first_sub=None):
                r = rawp.tile([96, GK, W], bf16, tag=f"raw{g}")
                e = exp_.tile([96, GK, W], bf16, tag=f"ex{g}")
                sub = GK // split
                for s in range(split):
                    sl = slice(s * sub, (s + 1) * sub)
                    nc.sync.dma_start(
                        out=r[:, sl, :],
                        in_=emi[g, :, grp * GK + s * sub : grp * GK + (s + 1) * sub, :],
                    )
                    nc.scalar.activation(
                        e[:, sl, :].rearrange("p a b -> p (a b)"),
                        r[:, sl, :].rearrange("p a b -> p (a b)"),
                        AF.Exp,
                    )
                    if s == 0 and on_first_sub is not None:
                        on_first_sub(r)
                raw[g][grp] = r
                ex[g][grp] = e

            def load_h(g, grp):
                h = ohp.tile([96, GK, W], bf16, tag=f"oh{g}")
                nc.sync.dma_start(
                    out=h[:], in_=hoh[g, :, grp * GK : (grp + 1) * GK, :]
                )
                oh[g][grp] = h
                # numerator em-at-tags partial: multiply on the idle gpsimd.
                # The scalar-engine reduce is deferred one group so it never
                # delays the next emission exp in the in-order scalar queue.
                k = nacc["k"]
                p = prd.tile([96, GK * W], bf16, tag="prod")
                nc.gpsimd.tensor_tensor(
                    out=p[:],
                    in0=raw[g][grp][:].rearrange("p a b -> p (a b)"),
                    in1=h[:].rearrange("p a b -> p (a b)"),
                    op=OP.mult,
                )
                pend.append((p, k))
                nacc["k"] = k + 1

            def flush_acts():
                while pend:
                    p, k = pend.pop(0)
                    scr = scrp.tile([96, GK * W], bf16, tag="scr")
                    nc.scalar.activation(
                        scr[:], p[:], AF.Identity, accum_out=pacs[:, k : k + 1]
                    )

            # ---- scan ----
            # group 0 raw loads are split so the first exp lands early; the
            # chain-init activation is injected right after each chain's
            # first sub-exp so the first matmuls start ~10us in
            gcur = [None, None]

            def make_init(g):
                def _init(r):
                    # init: fwd row-block = exp(em_0 + trans[START,:]),
                    #       bwd row-block = exp(em_511 + trans[:,END])
                    g0 = gam.tile([96, W], bf16, tag=f"g{g}")
                    nc.scalar.activation(
                        g0[:], r[:, 0, :], AF.Exp, bias=ib_sb[:], scale=1.0
                    )
                    gcur[g] = g0
                return _init

            for g in range(NG):
                load_raw(g, 0, split=4, on_first_sub=make_init(g))
            for g in range(NG):
                load_h(g, 0)
            # small tail-phase constants: post now so their completion is not
            # queued behind the whole emission stream
            trn_sb = const.tile([TT2, TT2], f32, tag="trn")
            nc.sync.dma_start(out=trn_sb[:], in_=trn)
            cnt_sb = const.tile([TT2, TT2], f32, tag="cnt")
            nc.sync.dma_start(out=cnt_sb[:], in_=cnt)

            for t in range(1, HT):
                grp, slot = divmod(t, GK)
                if slot == 0 and raw[0][grp] is None:
                    for g in range(NG):
                        load_raw(g, grp)
                    for g in range(NG):
                        load_h(g, grp)
                    flush_acts()
                for g in range(NG):
                    ps = psp.tile([96, W], f32, tag=f"ps{g}")
                    mm = nc.tensor.matmul(
                        ps[:], w_sb[:], gcur[g][:], start=True, stop=True
                    )
                    if t > 1:
                        # weights stay resident in the PE array; only the
                        # first matmul of each chain self-loads them.
                        mm.ins.ldweights = False
                    gn = gam.tile([96, W], bf16, tag=f"g{g}")
                    nc.vector.tensor_mul(
                        out=gn[:], in0=ps[:], in1=ex[g][grp][:, slot, :]
                    )
                    gcur[g] = gn

            flush_acts()

            # ---- meet in the middle:  Z*c^511 = (cA^T f_255)^T ghat_256 ----
            zt = fin.tile([48, NG * W], f32, tag="zt")
            for g in range(NG):
                psm = psf.tile([96, W], f32, tag=f"meet{g}")
                mm = nc.tensor.matmul(
                    psm[:], w_sb[:], gcur[g][:], start=True, stop=True
                )
                mm.ins.ldweights = False
                gmv = fin.tile([48, W], bf16, tag=f"gmv{g}")
                nc.sync.dma_start(out=gmv[:], in_=gcur[g][48:96, :])
                nc.vector.tensor_mul(
                    out=zt[:, g * W : (g + 1) * W], in0=psm[0:48, :], in1=gmv[:]
                )
            psz = psf.tile([1, NG * W], f32, tag="psz")
            nc.tensor.matmul(psz[:], ones48[:], zt[:], start=True, stop=True)
            dsb = fin.tile([1, NG * W], f32, tag="dsb")
            nc.scalar.activation(dsb[:], psz[:], AF.Ln)
            dred = fin.tile([1, 1], f32, tag="dred")
            nc.vector.tensor_reduce(dred[:], dsb[:], axis=AX.X, op=OP.add)
            nc.sync.dma_start(out=den, in_=dred[:])

            # ---- numerator finalization ----
            # transitions part: dot(trn, cnt) -> per-partition sums tac[50,1]
            tscr = fin.tile([TT2, TT2], f32, tag="tscr")
            tac = fin.tile([TT2, 1], f32, tag="tac")
            nc.vector.scalar_tensor_tensor(
                out=tscr[:],
                in0=trn_sb[:],
                scalar=1.0,
                in1=cnt_sb[:],
                op0=OP.mult,
                op1=OP.mult,
                accum_out=tac[:],
            )
            pvr = fin.tile([96, 1], f32, tag="pvr")
            nc.vector.tensor_reduce(pvr[:], pacs[:], axis=AX.X, op=OP.add)
            psn = psf.tile([1, 2], f32, tag="psn")
            nc.tensor.matmul(psn[:, 0:1], ones96[:], pvr[:], start=True, stop=True)
            nc.tensor.matmul(psn[:, 1:2], ones50[:], tac[:], start=True, stop=True)
            # num = em-at-tags total + transitions dot
            pr = fin.tile([1, 2], f32, tag="pr")
            nc.vector.tensor_copy(out=pr[:], in_=psn[:])
            nsb = fin.tile([1, 1], f32, tag="nsb")
            nc.vector.tensor_reduce(nsb[:], pr[:], axis=AX.X, op=OP.add)
            nc.sync.dma_start(out=num, in_=nsb[:])

    nc.compile()
    return nc


def _prep(emissions, tags, transitions):
    import ml_dtypes

    bf16 = ml_dtypes.bfloat16
    em16 = np.asarray(emissions).astype(bf16)
    tg = np.ascontiguousarray(tags).astype(np.int32)
    tr = np.ascontiguousarray(transitions, dtype=np.float32)

    A = np.exp(tr[:T, :T].astype(np.float64))
    c = 1.0 / (A.sum(axis=0).mean() * np.exp(0.5))
    logc = float(np.log(c))
    cA = c * A
    w1 = np.zeros((96, 96), np.float64)
    w1[:48, :48] = cA
    w1[48:, 48:] = cA.T
    wmat = np.ascontiguousarray(w1, dtype=bf16)
    initb = np.ascontiguousarray(
        np.concatenate([tr[T, :T], tr[:T, T + 1]])[:, None], dtype=np.float32
    )

    jj = np.arange(T, dtype=np.int32)
    mask = tg != -1
    safe = np.where(mask, tg, -2)  # never matches a real tag
    last_idx = mask.sum(axis=1) - 1

    in_maps = []
    for core in range(NCORES):
        b0 = core * BL
        em_c = em16[b0 : b0 + BL]                        # [128, 512, 48]
        tg_c = safe[b0 : b0 + BL]                        # [128, 512]
        fwd = em_c[:, :HT, :]                            # [128, 256, 48]
        bwd = em_c[:, S - 1 : HT - 1 : -1, :]            # s = 511..256
        # one-hot in [j, t, w] layout
        fH = (tg_c[:, :HT].T[None, :, :] == jj[:, None, None])
        bH = (tg_c[:, S - 1 : HT - 1 : -1].T[None, :, :] == jj[:, None, None])
        emi = np.empty((NG, 96, HT, W), bf16)
        hohv = np.empty((NG, 96, HT, W), bf16)
        for g in range(NG):
            cols = slice(g * W, (g + 1) * W)
            emi[g, :48] = fwd[cols].transpose(2, 1, 0)
            emi[g, 48:] = bwd[cols].transpose(2, 1, 0)
            hohv[g, :48] = fH[:, :, cols].astype(bf16)
            hohv[g, 48:] = bH[:, :, cols].astype(bf16)

        # transition count matrix (START row 48, END col 49)
        tgc = np.clip(tg[b0 : b0 + BL], 0, None)
        m_c = mask[b0 : b0 + BL]
        cntv = np.zeros(TT2 * TT2, np.float64)
        cntv += np.bincount(T * TT2 + tgc[:, 0], minlength=TT2 * TT2)
        pair = tgc[:, :-1] * TT2 + tgc[:, 1:]
        valid = m_c[:, 1:]
        cntv += np.bincount(pair[valid].ravel(), minlength=TT2 * TT2)
        lt = tgc[np.arange(BL), last_idx[b0 : b0 + BL]]
        cntv += np.bincount(lt * TT2 + (T + 1), minlength=TT2 * TT2)

        in_maps.append(
            {
                "emi": emi,
                "hoh": hohv,
                "trn": tr.reshape(TT2, TT2),
                "cnt": np.ascontiguousarray(
                    cntv.reshape(TT2, TT2), dtype=np.float32
                ),
                "wmat": wmat,
                "initb": initb,
            }
        )
    return in_maps, logc


def kernel(emissions, tags, transitions):
    from concourse.bass_utils import run_bass_kernel_spmd

    if "nc" not in _CACHE:
        _CACHE["nc"] = _build_module()
    nc = _CACHE["nc"]

    in_maps, logc = _prep(emissions, tags, transitions)
    res = run_bass_kernel_spmd(
        nc, in_maps, core_ids=list(range(NCORES)), trace=_TRACE
    )
    LAST["exec_ns"] = res.exec_time_ns
    LAST["results"] = res.results
    LAST["trace"] = res.instructions_and_trace

    total = 0.0
    for core in range(NCORES):
        r = res.results[core]
        d = float(r["den"].reshape(()))     # sum_b ln(c^511 * Z_b)
        n = float(r["num"].reshape(()))
        total += n - (d - BL * 511.0 * logc)
    return np.asarray(total / B, dtype=np.float32)


# revision 19
# speedup vs baseline: 1.7649x; 1.0068x over previous
"""CRF loss (forward-algorithm log-partition + joint LLH) on 8 Trainium2 cores.

Strategy
--------
Data parallel over batch: each of the 8 cores handles 128 batch rows.

Denominator (log-partition): the 512-step forward scan runs in *scaled
probability space* so each step is one small PE matmul plus one DVE
tensor-tensor multiply:

    gamma_{s+1} = (c*A)^T gamma_s  (*) E_{s+1},   E_s = exp(emissions_s)

A = exp(Ttt) is the 48x48 transition kernel; the overflow-control constant
c is folded into the bf16 matmul weights. The scan runs forward from s=0
and backward from s=511 simultaneously (meet in the middle at s=255/256),
stacked in one [96, 64] tile: partitions 0-47 forward, 48-95 backward,
with blockdiag(cA, (cA)^T) weights. Two such fused chains (batch columns
0-63 and 64-127) interleave to hide cross-engine latency. Emissions ship
as bf16 in a partition-major [chain, 96, tick, 64] layout so each group
DMA is 96 contiguous 4KB descriptors.

Numerator (joint LLH): only the per-core *sum* is needed (the output is a
scalar mean), so
  - the transition term collapses to dot(transitions, count_matrix) where
    the 50x50 count matrix (incl. START row / END col) is a host-side
    tags transform, and
  - the emission-at-tags term is sum(raw_em (*) onehot(tags)) computed by
    one fused DVE tensor_tensor_reduce per emission group against the
    scan's already-resident tiles (one-hot ships as bf16 in the same
    layout). Both collapse into a single PSUM accumulation.

Host does only: sharding, layout transforms, count/one-hot encoding of
tags, and the final mean over the 8 per-core scalars (all-reduce glue)
plus the +511*ln(1/c) constant.
"""

import numpy as np

B, S, T = 1024, 512, 48
TT2 = T + 2                 # 50: table side incl. START/END
NCORES = 8
BL = B // NCORES            # 128 batch rows per core
NG = 2                      # fused chains per core (64 batch cols each)
W = 64                      # batch columns per chain
HT = 256                    # tick 0 = init, ticks 1..255 = scan, meet after
GK = 32                     # ticks per emission super-tile (DMA/exp batch)
NGRP = HT // GK             # 8 groups per chain

_CACHE = {}
_TRACE = False
_NUM_ENGINE = "gpsimd"      # "vector" (DVE TTR) or "gpsimd" (fused STT)
LAST = {"exec_ns": None, "results": None, "trace": None}


def _build_module():
    from concourse import bacc
    import concourse.mybir as mybir
    import concourse.tile as tile

    f32 = mybir.dt.float32
    bf16 = mybir.dt.bfloat16

    nc = bacc.Bacc(
        "TRN2",
        target_bir_lowering=False,
        debug=False,
        enable_asserts=False,
    )

    emi = nc.dram_tensor("emi", [NG, 96, HT, W], bf16, kind="ExternalInput").ap()
    hoh = nc.dram_tensor("hoh", [NG, 96, HT, W], bf16, kind="ExternalInput").ap()
    trn = nc.dram_tensor("trn", [TT2, TT2], f32, kind="ExternalInput").ap()
    cnt = nc.dram_tensor("cnt", [TT2, TT2], f32, kind="ExternalInput").ap()
    wmat = nc.dram_tensor("wmat", [96, 96], bf16, kind="ExternalInput").ap()
    initb = nc.dram_tensor("initb", [96, 1], f32, kind="ExternalInput").ap()
    den = nc.dram_tensor("den", [1, 1], f32, kind="ExternalOutput").ap()
    num = nc.dram_tensor("num", [1, 1], f32, kind="ExternalOutput").ap()

    AF = mybir.ActivationFunctionType
    OP = mybir.AluOpType
    AX = mybir.AxisListType

    with tile.TileContext(nc) as tc:
        with (
            tc.tile_pool(name="const", bufs=1) as const,
            tc.tile_pool(name="raw", bufs=4) as rawp,
            tc.tile_pool(name="ex", bufs=4) as exp_,
            tc.tile_pool(name="oh", bufs=4) as ohp,
            tc.tile_pool(name="gam", bufs=3) as gam,
            tc.tile_pool(name="prd", bufs=3) as prd,
            tc.tile_pool(name="scr", bufs=2) as scrp,
            tc.tile_pool(name="fin", bufs=1) as fin,
            tc.tile_pool(name="ps", bufs=2, space="PSUM") as psp,
            tc.tile_pool(name="psfin", bufs=1, space="PSUM") as psf,
        ):
            # ---- constants ----
            w_sb = const.tile([96, 96], bf16, tag="w")
            ib_sb = const.tile([96, 1], f32, tag="ib")
            ones48 = const.tile([48, 1], f32, tag="ones48")
            nc.vector.memset(ones48[:], 1.0)
            ones96 = const.tile([96, 1], f32, tag="ones96")
            nc.vector.memset(ones96[:], 1.0)
            ones50 = const.tile([TT2, 1], f32, tag="ones50")
            nc.vector.memset(ones50[:], 1.0)
            # numerator accumulator: pacs[:, k] = per-partition em-sums of
            # group k (gpsimd multiplies, scalar engine reduces via accum_out)
            pacs = const.tile([96, NG * NGRP], f32, tag="pacs")

            raw = [[None] * NGRP for _ in range(NG)]
            ex = [[None] * NGRP for _ in range(NG)]
            oh = [[None] * NGRP for _ in range(NG)]
            nacc = {"k": 0}
            pend = []

            def load_raw(g, grp, split=1, on_first_sub=None):
                r = rawp.tile([96, GK, W], bf16, tag=f"raw{g}")
                e = exp_.tile([96, GK, W], bf16, tag=f"ex{g}")
                sub = GK // split
                for s in range(split):
                    sl = slice(s * sub, (s + 1) * sub)
                    nc.sync.dma_start(
                        out=r[:, sl, :],
                        in_=emi[g, :, grp * GK + s * sub : grp * GK + (s + 1) * sub, :],
                    )
                    nc.scalar.activation(
                        e[:, sl, :].rearrange("p a b -> p (a b)"),
                        r[:, sl, :].rearrange("p a b -> p (a b)"),
                        AF.Exp,
                    )
                    if s == 0 and on_first_sub is not None:
                        on_first_sub(r)
                raw[g][grp] = r
                ex[g][grp] = e

            def load_h(g, grp):
                h = ohp.tile([96, GK, W], bf16, tag=f"oh{g}")
                nc.sync.dma_start(
                    out=h[:], in_=hoh[g, :, grp * GK : (grp + 1) * GK, :]
                )
                oh[g][grp] = h
                # numerator em-at-tags partial: multiply on the idle gpsimd.
                # The scalar-engine reduce is deferred one group so it never
                # delays the next emission exp in the in-order scalar queue.
                k = nacc["k"]
                p = prd.tile([96, GK * W], bf16, tag="prod")
                nc.gpsimd.tensor_tensor(
                    out=p[:],
                    in0=raw[g][grp][:].rearrange("p a b -> p (a b)"),
                    in1=h[:].rearrange("p a b -> p (a b)"),
                    op=OP.mult,
                )
                pend.append((p, k))
                nacc["k"] = k + 1

            def flush_acts():
                while pend:
                    p, k = pend.pop(0)
                    scr = scrp.tile([96, GK * W], bf16, tag="scr")
                    nc.scalar.activation(
                        scr[:], p[:], AF.Identity, accum_out=pacs[:, k : k + 1]
                    )

            # ---- scan ----
            # group 0 raw loads are split so the first exp lands early; the
            # chain-init activation is injected right after each chain's
            # first sub-exp so the first matmuls start ~10us in
            gcur = [None, None]

            def make_init(g):
                def _init(r):
                    # init: fwd row-block = exp(em_0 + trans[START,:]),
                    #       bwd row-block = exp(em_511 + trans[:,END])
                    g0 = gam.tile([96, W], bf16, tag=f"g{g}")
                    nc.scalar.activation(
                        g0[:], r[:, 0, :], AF.Exp, bias=ib_sb[:], scale=1.0
                    )
                    gcur[g] = g0
                return _init

            def init_and_consts(g):
                base = make_init(g)
                def _f(r):
                    if g == 0:
                        # needed only ~10us in (first init / first matmul);
                        # posting after the first emission sub-DMA keeps the
                        # DMA queues clear for the ramp-critical load
                        nc.sync.dma_start(out=w_sb[:], in_=wmat)
                        nc.sync.dma_start(out=ib_sb[:], in_=initb)
                    base(r)
                return _f

            for g in range(NG):
                load_raw(g, 0, split=4, on_first_sub=init_and_consts(g))
            for g in range(NG):
                load_h(g, 0)
            # small tail-phase constants: post now so their completion is not
            # queued behind the whole emission stream
            trn_sb = const.tile([TT2, TT2], f32, tag="trn")
            nc.sync.dma_start(out=trn_sb[:], in_=trn)
            cnt_sb = const.tile([TT2, TT2], f32, tag="cnt")
            nc.sync.dma_start(out=cnt_sb[:], in_=cnt)

            for t in range(1, HT):
                grp, slot = divmod(t, GK)
                if slot == 0 and raw[0][grp] is None:
                    for g in range(NG):
                        load_raw(g, grp)
                    for g in range(NG):
                        load_h(g, grp)
                    flush_acts()
                for g in range(NG):
                    ps = psp.tile([96, W], f32, tag=f"ps{g}")
                    mm = nc.tensor.matmul(
                        ps[:], w_sb[:], gcur[g][:], start=True, stop=True
                    )
                    if t > 1:
                        # weights stay resident in the PE array; only the
                        # first matmul of each chain self-loads them.
                        mm.ins.ldweights = False
                    gn = gam.tile([96, W], bf16, tag=f"g{g}")
                    nc.vector.tensor_mul(
                        out=gn[:], in0=ps[:], in1=ex[g][grp][:, slot, :]
                    )
                    gcur[g] = gn

            flush_acts()

            # ---- meet in the middle:  Z*c^511 = (cA^T f_255)^T ghat_256 ----
            zt = fin.tile([48, NG * W], f32, tag="zt")
            for g in range(NG):
                psm = psf.tile([96, W], f32, tag=f"meet{g}")
                mm = nc.tensor.matmul(
                    psm[:], w_sb[:], gcur[g][:], start=True, stop=True
                )
                mm.ins.ldweights = False
                gmv = fin.tile([48, W], bf16, tag=f"gmv{g}")
                nc.sync.dma_start(out=gmv[:], in_=gcur[g][48:96, :])
                nc.vector.tensor_mul(
                    out=zt[:, g * W : (g + 1) * W], in0=psm[0:48, :], in1=gmv[:]
                )
            psz = psf.tile([1, NG * W], f32, tag="psz")
            nc.tensor.matmul(psz[:], ones48[:], zt[:], start=True, stop=True)
            dsb = fin.tile([1, NG * W], f32, tag="dsb")
            nc.scalar.activation(dsb[:], psz[:], AF.Ln)
            dred = fin.tile([1, 1], f32, tag="dred")
            nc.vector.tensor_reduce(dred[:], dsb[:], axis=AX.X, op=OP.add)
            nc.sync.dma_start(out=den, in_=dred[:])

            # ---- numerator finalization ----
            # transitions part: dot(trn, cnt) -> per-partition sums tac[50,1]
            tscr = fin.tile([TT2, TT2], f32, tag="tscr")
            tac = fin.tile([TT2, 1], f32, tag="tac")
            # WAW fence on tscr: ties the transition dot behind the scan's
            # final LN so the scheduler cannot hoist it into mid-scan DVE
            # gaps (where it would block on its input DMAs)
            nc.vector.tensor_copy(out=tscr[0:1, 0:1], in_=dred[:])
            nc.vector.scalar_tensor_tensor(
                out=tscr[:],
                in0=trn_sb[:],
                scalar=1.0,
                in1=cnt_sb[:],
                op0=OP.mult,
                op1=OP.mult,
                accum_out=tac[:],
            )
            pvr = fin.tile([96, 1], f32, tag="pvr")
            nc.vector.tensor_reduce(pvr[:], pacs[:], axis=AX.X, op=OP.add)
            psn = psf.tile([1, 2], f32, tag="psn")
            nc.tensor.matmul(psn[:, 0:1], ones96[:], pvr[:], start=True, stop=True)
            nc.tensor.matmul(psn[:, 1:2], ones50[:], tac[:], start=True, stop=True)
            # num = em-at-tags total + transitions dot
            pr = fin.tile([1, 2], f32, tag="pr")
            nc.vector.tensor_copy(out=pr[:], in_=psn[:])
            nsb = fin.tile([1, 1], f32, tag="nsb")
            nc.vector.tensor_reduce(nsb[:], pr[:], axis=AX.X, op=OP.add)
            nc.sync.dma_start(out=num, in_=nsb[:])

    nc.compile()
    return nc


def _prep(emissions, tags, transitions):
    import ml_dtypes

    bf16 = ml_dtypes.bfloat16
    em16 = np.asarray(emissions).astype(bf16)
    tg = np.ascontiguousarray(tags).astype(np.int32)
    tr = np.ascontiguousarray(transitions, dtype=np.float32)

    A = np.exp(tr[:T, :T].astype(np.float64))
    c = 1.0 / (A.sum(axis=0).mean() * np.exp(0.5))
    logc = float(np.log(c))
    cA = c * A
    w1 = np.zeros((96, 96), np.float64)
    w1[:48, :48] = cA
    w1[48:, 48:] = cA.T
    wmat = np.ascontiguousarray(w1, dtype=bf16)
    initb = np.ascontiguousarray(
        np.concatenate([tr[T, :T], tr[:T, T + 1]])[:, None], dtype=np.float32
    )

    jj = np.arange(T, dtype=np.int32)
    mask = tg != -1
    safe = np.where(mask, tg, -2)  # never matches a real tag
    last_idx = mask.sum(axis=1) - 1

    in_maps = []
    for core in range(NCORES):
        b0 = core * BL
        em_c = em16[b0 : b0 + BL]                        # [128, 512, 48]
        tg_c = safe[b0 : b0 + BL]                        # [128, 512]
        fwd = em_c[:, :HT, :]                            # [128, 256, 48]
        bwd = em_c[:, S - 1 : HT - 1 : -1, :]            # s = 511..256
        # one-hot in [j, t, w] layout
        fH = (tg_c[:, :HT].T[None, :, :] == jj[:, None, None])
        bH = (tg_c[:, S - 1 : HT - 1 : -1].T[None, :, :] == jj[:, None, None])
        emi = np.empty((NG, 96, HT, W), bf16)
        hohv = np.empty((NG, 96, HT, W), bf16)
        for g in range(NG):
            cols = slice(g * W, (g + 1) * W)
            emi[g, :48] = fwd[cols].transpose(2, 1, 0)
            emi[g, 48:] = bwd[cols].transpose(2, 1, 0)
            hohv[g, :48] = fH[:, :, cols].astype(bf16)
            hohv[g, 48:] = bH[:, :, cols].astype(bf16)

        # transition count matrix (START row 48, END col 49)
        tgc = np.clip(tg[b0 : b0 + BL], 0, None)
        m_c = mask[b0 : b0 + BL]
        cntv = np.zeros(TT2 * TT2, np.float64)
        cntv += np.bincount(T * TT2 + tgc[:, 0], minlength=TT2 * TT2)
        pair = tgc[:, :-1] * TT2 + tgc[:, 1:]
        valid = m_c[:, 1:]
        cntv += np.bincount(pair[valid].ravel(), minlength=TT2 * TT2)
        lt = tgc[np.arange(BL), last_idx[b0 : b0 + BL]]
        cntv += np.bincount(lt * TT2 + (T + 1), minlength=TT2 * TT2)

        in_maps.append(
            {
                "emi": emi,
                "hoh": hohv,
                "trn": tr.reshape(TT2, TT2),
                "cnt": np.ascontiguousarray(
                    cntv.reshape(TT2, TT2), dtype=np.float32
                ),
                "wmat": wmat,
                "initb": initb,
            }
        )
    return in_maps, logc


def kernel(emissions, tags, transitions):
    from concourse.bass_utils import run_bass_kernel_spmd

    if "nc" not in _CACHE:
        _CACHE["nc"] = _build_module()
    nc = _CACHE["nc"]

    in_maps, logc = _prep(emissions, tags, transitions)
    res = run_bass_kernel_spmd(
        nc, in_maps, core_ids=list(range(NCORES)), trace=_TRACE
    )
    LAST["exec_ns"] = res.exec_time_ns
    LAST["results"] = res.results
    LAST["trace"] = res.instructions_and_trace

    total = 0.0
    for core in range(NCORES):
        r = res.results[core]
        d = float(r["den"].reshape(()))     # sum_b ln(c^511 * Z_b)
        n = float(r["num"].reshape(()))
        total += n - (d - BL * 511.0 * logc)
    return np.asarray(total / B, dtype=np.float32)
